# revision 11
# baseline (speedup 1.0000x reference)
"""DLRM forward (embedding gather + tiny MLPs) as a Bass/Tile kernel on 8 trn2 cores.

Sharding: data-parallel over the batch. Each of the 8 cores gets B/8 = 2048 rows
of dense_x / sparse_x plus a full replica of the (read-only) embedding tables,
computes its batch shard end-to-end on device, and returns [1, 2048] sigmoid
outputs. The host only slices inputs and concatenates outputs.

v2 design (vs v1: fp16 tables + per-128-tile gathers + PE transposes):
  - tables quantized host-side to fp8e4 (x256 scale) -> 64B gather rows, half
    the HBM gather traffic; scale undone in the o1 activation (ALPHA).
  - flat gather indices (idx + f*CARD) precomputed on host, padded to 28
    rows/sample so each 128-sample frame is 1792B = 896 u16 (xbar-friendly).
  - one indirect DMA per 512-sample group (4 total) -> fewer SWDGE gens.
  - feature->partition transposes done by the DMA XBAR (dma_start
    transpose=True) on uint16 views: one instruction block-transposes
    [128 samples, 7x128 u16] -> [128 u16-feat, 7, 128 samples]; each u16
    carries an (even,odd) fp8 feature pair, which is exactly the DoubleRow
    matmul operand layout.
  - top-MLP layer 1 accumulated with fp8 DoubleRow matmuls (2 features per
    partition per pass), weights host-packed into [q, c, r, m] order.
  - dense_x host-transposed to [13, B] fp16 so the bottom MLP needs no PE
    transposes at all.
"""

import numpy as np
import ml_dtypes

import concourse.bass as bass
import concourse.mybir as mybir
import concourse.tile as tile
from concourse import bacc

P = 128

# Problem constants (hardcoded per harness contract).
N_CORES = 8
B = 16384
F = 26
D = 64
DENSE = 13
CARD = 100000
H_BOT = 8
H_TOP = 16

FPAD = 28                 # gathered rows per sample (26 + 2 pad -> 1792B frames)
FRAME = FPAD * D          # 1792 fp8 per sample-frame
KC = FRAME // 2 // P      # 7 u16 feature chunks of 128
WPACK = 644               # packed small-weights blob bytes per partition

S_T = 256.0               # host scale on tables before fp8e4 quantization
S_W = 64.0                # host scale on tw1 emb rows before fp8e4 quantization
ALPHA = 1.0 / (S_T * S_W)

f32 = mybir.dt.float32
i32 = mybir.dt.int32
fp16 = mybir.dt.float16
u16 = mybir.dt.uint16
f8 = mybir.dt.float8e4

USE_DOUBLE_ROW = True


def build_kernel(b_loc=B // N_CORES):
    group = 512
    n_g = b_loc // group
    tpg = group // P

    nc = bacc.Bacc("TRN2", target_bir_lowering=False)
    comb_d = nc.dram_tensor("comb", [P, n_g * tpg * FPAD], i32, kind="ExternalInput")
    tables_d = nc.dram_tensor("tables", [F * CARD, D], f8, kind="ExternalInput")
    dxt_d = nc.dram_tensor("dxt", [DENSE, b_loc], fp16, kind="ExternalInput")
    wpk_d = nc.dram_tensor("wpk", [P, WPACK], mybir.dt.uint8, kind="ExternalInput")
    y_d = nc.dram_tensor("y", [1, b_loc], f32, kind="ExternalOutput")

    with tile.TileContext(nc) as tc:
        with (
            tc.tile_pool(name="const", bufs=1) as cpool,
            tc.tile_pool(name="emb", bufs=2) as embp,
            tc.tile_pool(name="embT", bufs=2) as embtp,
            tc.tile_pool(name="small", bufs=2) as smallp,
            tc.tile_pool(name="po1", bufs=2, space="PSUM") as po1p,
            tc.tile_pool(name="psmall", bufs=2, space="PSUM") as psmallp,
        ):
            # ---- constants / inputs staged once ----
            # comb first (gates the gathers); all small weights arrive in one
            # packed u8 DMA and are sliced out as bitcast views.
            comb_sb = cpool.tile([P, n_g * tpg * FPAD], i32)
            nc.sync.dma_start(out=comb_sb[:], in_=comb_d[:, :])
            wpk_sb = cpool.tile([P, WPACK], mybir.dt.uint8)
            nc.sync.dma_start(out=wpk_sb[:], in_=wpk_d[:, :])
            dxt_sb = cpool.tile([DENSE, b_loc], fp16)
            nc.sync.dma_start(out=dxt_sb[:], in_=dxt_d[:, :])

            tw1dr_sb = wpk_sb[:, 0:448].bitcast(f8)
            w1_sb = wpk_sb[0:DENSE, 448:464].bitcast(fp16)
            w2_sb = wpk_sb[0:H_BOT, 464:592].bitcast(fp16)
            tw1d_sb = wpk_sb[0:D, 592:624].bitcast(fp16)
            tw2_sb = wpk_sb[0:H_TOP, 624:626].bitcast(fp16)
            b1_sb = wpk_sb[0:H_BOT, 628:632].bitcast(f32)
            b2_sb = wpk_sb[0:D, 632:636].bitcast(f32)
            tb1_sb = wpk_sb[0:H_TOP, 636:640].bitcast(f32)
            tb2_sb = wpk_sb[0:1, 640:644].bitcast(f32)

            y_row = cpool.tile([1, b_loc], f32)

            for g in range(n_g):
                # ---- gather: one indirect DMA for the whole 512-sample group ----
                emb_g = embp.tile([P, tpg * FRAME], f8, tag="emb")
                nc.gpsimd.indirect_dma_start(
                    out=emb_g[:],
                    out_offset=None,
                    in_=tables_d[:, :],
                    in_offset=bass.IndirectOffsetOnAxis(
                        ap=comb_sb[:, bass.ts(g, tpg * FPAD)], axis=0
                    ),
                )

                # ---- feature->partition transpose via DMA XBAR on u16 views ----
                # one instruction block-transposes the whole group's 28
                # [128,128]-u16 blocks: in [128 samples, 3584 u16] ->
                # out [128 u16-feat, 28 (j,c) blocks, 128 samples]
                embt_g = embtp.tile([P, tpg, KC, P], u16, tag="embT")
                eng = nc.sync if g % 2 == 0 else nc.scalar
                eng.dma_start(
                    out=embt_g[:],
                    in_=emb_g[:].bitcast(u16),
                    transpose=True,
                )

                # ---- bottom MLP (host-transposed dense, fp16) ----
                ph = psmallp.tile([H_BOT, group], f32, tag="psmall")
                nc.tensor.matmul(
                    out=ph[:], lhsT=w1_sb, rhs=dxt_sb[:, bass.ts(g, group)],
                    start=True, stop=True,
                )
                h_s = smallp.tile([H_BOT, group], fp16, tag="h")
                nc.scalar.activation(
                    out=h_s[:], in_=ph[:],
                    func=mybir.ActivationFunctionType.Relu, bias=b1_sb,
                )
                pd = psmallp.tile([D, group], f32, tag="psmall")
                nc.tensor.matmul(
                    out=pd[:], lhsT=w2_sb, rhs=h_s[:], start=True, stop=True
                )
                dt_sb = smallp.tile([D, group], fp16, tag="dt")
                nc.scalar.activation(
                    out=dt_sb[:], in_=pd[:],
                    func=mybir.ActivationFunctionType.Identity, bias=b2_sb,
                )

                # ---- top MLP layer 1: fp8 DoubleRow accumulation ----
                po1 = po1p.tile([H_TOP, group], f32, tag="po1")
                for c in range(KC):
                    lhs = tw1dr_sb[:, bass.ts(c, 2 * H_TOP)].rearrange(
                        "p (r m) -> p r m", r=2
                    )
                    rhs = embt_g[:, :, c, :].bitcast(f8).rearrange(
                        "p j (s r) -> p r j s", r=2
                    )
                    if USE_DOUBLE_ROW:
                        nc.tensor.matmul(
                            out=po1[:], lhsT=lhs, rhs=rhs,
                            start=(c == 0), stop=False,
                            perf_mode=mybir.MatmulPerfMode.DoubleRow,
                            skip_group_check=True,
                        )
                    else:
                        for r in range(2):
                            nc.tensor.matmul(
                                out=po1[:], lhsT=lhs[:, r, :], rhs=rhs[:, r],
                                start=(c == 0 and r == 0), stop=False,
                            )
                # dense chunk carries the same S_T*S_W scale (folded into tw1d)
                nc.tensor.matmul(
                    out=po1[:], lhsT=tw1d_sb, rhs=dt_sb[:],
                    start=False, stop=True,
                )

                o1 = smallp.tile([H_TOP, group], fp16, tag="o1")
                nc.scalar.activation(
                    out=o1[:], in_=po1[:],
                    func=mybir.ActivationFunctionType.Relu,
                    bias=tb1_sb, scale=ALPHA,
                )
                plg = psmallp.tile([1, group], f32, tag="psmall")
                nc.tensor.matmul(
                    out=plg[:], lhsT=tw2_sb, rhs=o1[:], start=True, stop=True
                )
                nc.scalar.activation(
                    out=y_row[:, bass.ts(g, group)], in_=plg[:],
                    func=mybir.ActivationFunctionType.Sigmoid, bias=tb2_sb,
                )

            nc.sync.dma_start(out=y_d[:, :], in_=y_row[:])

    nc.compile()
    return nc


_NC_CACHE = {}


def _get_nc():
    if "nc" not in _NC_CACHE:
        _NC_CACHE["nc"] = build_kernel()
    return _NC_CACHE["nc"]


def make_in_maps(dense_x, sparse_x, tables, w1, b1, w2, b2, tw1, tb1, tw2, tb2):
    b_loc = B // N_CORES
    group = 512
    n_g = b_loc // group
    tpg = group // P

    tables_q = np.ascontiguousarray(
        (np.asarray(tables, np.float32).reshape(F * CARD, D) * S_T).astype(
            ml_dtypes.float8_e4m3
        )
    )

    tw1_f = np.asarray(tw1, np.float32)
    tw1s = np.zeros((KC * 2 * P, H_TOP), np.float32)
    tw1s[: F * D] = tw1_f[: F * D] * S_W
    # [k, m] -> [c, q, r, m] -> [q, c, r, m]; k = c*256 + q*2 + r
    tw1dr = (
        tw1s.reshape(KC, P, 2, H_TOP)
        .transpose(1, 0, 2, 3)
        .reshape(P, KC * 2 * H_TOP)
        .astype(ml_dtypes.float8_e4m3)
    )
    tw1d = np.ascontiguousarray(
        (tw1_f[F * D : F * D + D] * (S_T * S_W)).astype(np.float16)
    )

    dense_f = np.asarray(dense_x, np.float32)
    sparse_i = np.asarray(sparse_x, np.int64).astype(np.int32)
    foffs = (np.arange(F, dtype=np.int32) * CARD)[None, :]

    wpk = np.zeros((P, WPACK), np.uint8)
    def put(arr, rows, c0):
        b = np.ascontiguousarray(arr).view(np.uint8).reshape(rows, -1)
        wpk[:rows, c0:c0 + b.shape[1]] = b
    put(tw1dr, P, 0)
    put(np.asarray(w1, np.float16), DENSE, 448)
    put(np.asarray(w2, np.float16), H_BOT, 464)
    put(tw1d, D, 592)
    put(np.asarray(tw2, np.float16), H_TOP, 624)
    put(np.asarray(b1, np.float32).reshape(H_BOT, 1), H_BOT, 628)
    put(np.asarray(b2, np.float32).reshape(D, 1), D, 632)
    put(np.asarray(tb1, np.float32).reshape(H_TOP, 1), H_TOP, 636)
    put(np.asarray(tb2, np.float32).reshape(1, 1), 1, 640)
    shared = {"tables": tables_q, "wpk": wpk}

    in_maps = []
    for cidx in range(N_CORES):
        sl = slice(cidx * b_loc, (cidx + 1) * b_loc)
        comb = np.zeros((b_loc, FPAD), np.int32)
        comb[:, :F] = sparse_i[sl] + foffs
        comb = np.ascontiguousarray(
            comb.reshape(n_g, tpg, P, FPAD)
            .transpose(2, 0, 1, 3)
            .reshape(P, n_g * tpg * FPAD)
        )
        dxt = np.ascontiguousarray(dense_f[sl].T.astype(np.float16))
        m = dict(shared)
        m["comb"] = comb
        m["dxt"] = dxt
        in_maps.append(m)
    return in_maps


def kernel(**inputs):
    from concourse.bass_utils import run_bass_kernel_spmd

    nc = _get_nc()
    in_maps = make_in_maps(**inputs)
    res = run_bass_kernel_spmd(nc, in_maps, core_ids=list(range(N_CORES)))
    out = np.concatenate([r["y"].reshape(-1) for r in res.results])
    return out.reshape(B, 1).astype(np.float32)


# revision 12
# speedup vs baseline: 1.1090x; 1.1090x over previous
"""DLRM forward (embedding gather + tiny MLPs) as a Bass/Tile kernel on 8 trn2 cores.

Sharding: data-parallel over the batch. Each of the 8 cores gets B/8 = 2048 rows
of dense_x / sparse_x plus a full replica of the (read-only) embedding tables,
computes its batch shard end-to-end on device, and returns [1, 2048] sigmoid
outputs. The host only slices inputs and concatenates outputs.

v2 design (vs v1: fp16 tables + per-128-tile gathers + PE transposes):
  - tables quantized host-side to fp8e4 (x256 scale) -> 64B gather rows, half
    the HBM gather traffic; scale undone in the o1 activation (ALPHA).
  - flat gather indices (idx + f*CARD) precomputed on host, padded to 28
    rows/sample so each 128-sample frame is 1792B = 896 u16 (xbar-friendly).
  - one indirect DMA per 512-sample group (4 total) -> fewer SWDGE gens.
  - feature->partition transposes done by the DMA XBAR (dma_start
    transpose=True) on uint16 views: one instruction block-transposes
    [128 samples, 7x128 u16] -> [128 u16-feat, 7, 128 samples]; each u16
    carries an (even,odd) fp8 feature pair, which is exactly the DoubleRow
    matmul operand layout.
  - top-MLP layer 1 accumulated with fp8 DoubleRow matmuls (2 features per
    partition per pass), weights host-packed into [q, c, r, m] order.
  - dense_x host-transposed to [13, B] fp16 so the bottom MLP needs no PE
    transposes at all.
"""

import numpy as np
import ml_dtypes

import concourse.bass as bass
import concourse.mybir as mybir
import concourse.tile as tile
from concourse import bacc

P = 128

# Problem constants (hardcoded per harness contract).
N_CORES = 8
B = 16384
F = 26
D = 64
DENSE = 13
CARD = 100000
H_BOT = 8
H_TOP = 16

FPAD = 28                 # gathered rows per sample (26 + 2 pad -> 1792B frames)
FRAME = FPAD * D          # 1792 fp8 per sample-frame
KC = FRAME // 2 // P      # 7 u16 feature chunks of 128

S_T = 256.0               # host scale on tables before fp8e4 quantization
S_W = 64.0                # host scale on tw1 emb rows before fp8e4 quantization
ALPHA = 1.0 / (S_T * S_W)

f32 = mybir.dt.float32
i32 = mybir.dt.int32
fp16 = mybir.dt.float16
u16 = mybir.dt.uint16
f8 = mybir.dt.float8e4

USE_DOUBLE_ROW = True


def build_kernel(b_loc=B // N_CORES):
    group = 512
    n_g = b_loc // group
    tpg = group // P

    nc = bacc.Bacc("TRN2", target_bir_lowering=False)
    comb_d = nc.dram_tensor("comb", [P, n_g * tpg * FPAD], i32, kind="ExternalInput")
    tables_d = nc.dram_tensor("tables", [F * CARD, D], f8, kind="ExternalInput")
    dxt_d = nc.dram_tensor("dxt", [DENSE, b_loc], fp16, kind="ExternalInput")
    w1_d = nc.dram_tensor("w1", [DENSE, H_BOT], fp16, kind="ExternalInput")
    b1_d = nc.dram_tensor("b1", [H_BOT], f32, kind="ExternalInput")
    w2_d = nc.dram_tensor("w2", [H_BOT, D], fp16, kind="ExternalInput")
    b2_d = nc.dram_tensor("b2", [D], f32, kind="ExternalInput")
    tw1dr_d = nc.dram_tensor("tw1dr", [P, KC * 2 * H_TOP], f8, kind="ExternalInput")
    tw1d_d = nc.dram_tensor("tw1d", [D, H_TOP], fp16, kind="ExternalInput")
    tb1_d = nc.dram_tensor("tb1", [H_TOP], f32, kind="ExternalInput")
    tw2_d = nc.dram_tensor("tw2", [H_TOP, 1], fp16, kind="ExternalInput")
    tb2_d = nc.dram_tensor("tb2", [1], f32, kind="ExternalInput")
    y_d = nc.dram_tensor("y", [1, b_loc], f32, kind="ExternalOutput")

    with tile.TileContext(nc) as tc:
        with (
            tc.tile_pool(name="const", bufs=1) as cpool,
            tc.tile_pool(name="emb", bufs=2) as embp,
            tc.tile_pool(name="embT", bufs=2) as embtp,
            tc.tile_pool(name="small", bufs=2) as smallp,
            tc.tile_pool(name="po1", bufs=2, space="PSUM") as po1p,
            tc.tile_pool(name="psmall", bufs=2, space="PSUM") as psmallp,
        ):
            # ---- constants / inputs staged once ----
            comb_sb = cpool.tile([P, n_g * tpg * FPAD], i32)
            nc.sync.dma_start(out=comb_sb[:], in_=comb_d[:, :])
            dxt_sb = cpool.tile([DENSE, b_loc], fp16)
            nc.sync.dma_start(out=dxt_sb[:], in_=dxt_d[:, :])
            tw1dr_sb = cpool.tile([P, KC * 2 * H_TOP], f8)
            nc.sync.dma_start(out=tw1dr_sb[:], in_=tw1dr_d[:, :])
            tw1d_sb = cpool.tile([D, H_TOP], fp16)
            nc.sync.dma_start(out=tw1d_sb[:], in_=tw1d_d[:, :])
            tw2_sb = cpool.tile([H_TOP, 1], fp16)
            nc.sync.dma_start(out=tw2_sb[:], in_=tw2_d[:, :])
            w1_sb = cpool.tile([DENSE, H_BOT], fp16)
            nc.sync.dma_start(out=w1_sb[:], in_=w1_d[:, :])
            w2_sb = cpool.tile([H_BOT, D], fp16)
            nc.sync.dma_start(out=w2_sb[:], in_=w2_d[:, :])
            b1_sb = cpool.tile([H_BOT, 1], f32)
            nc.sync.dma_start(out=b1_sb[:], in_=b1_d[:, None])
            b2_sb = cpool.tile([D, 1], f32)
            nc.sync.dma_start(out=b2_sb[:], in_=b2_d[:, None])
            tb1_sb = cpool.tile([H_TOP, 1], f32)
            nc.sync.dma_start(out=tb1_sb[:], in_=tb1_d[:, None])
            tb2_sb = cpool.tile([1, 1], f32)
            nc.sync.dma_start(out=tb2_sb[:], in_=tb2_d[:, None])

            y_row = cpool.tile([1, b_loc], f32)

            for g in range(n_g):
                # ---- gather: one indirect DMA for the whole 512-sample group ----
                emb_g = embp.tile([P, tpg * FRAME], f8, tag="emb")
                nc.gpsimd.indirect_dma_start(
                    out=emb_g[:],
                    out_offset=None,
                    in_=tables_d[:, :],
                    in_offset=bass.IndirectOffsetOnAxis(
                        ap=comb_sb[:, bass.ts(g, tpg * FPAD)], axis=0
                    ),
                )

                # ---- feature->partition transpose via DMA XBAR on u16 views ----
                # in:  [128 samples, 896 u16]  (u16 = fp8 feature pair)
                # out: [128 u16-feat, 7 chunks, 128 samples]
                embt_g = embtp.tile([P, KC, group], u16, tag="embT")
                for j in range(tpg):
                    eng = nc.sync if j % 2 == 0 else nc.scalar
                    eng.dma_start(
                        out=embt_g[:, :, bass.ts(j, P)],
                        in_=emb_g[:, bass.ts(j, FRAME)].bitcast(u16),
                        transpose=True,
                    )

                # ---- bottom MLP (host-transposed dense, fp16) ----
                ph = psmallp.tile([H_BOT, group], f32, tag="psmall")
                nc.tensor.matmul(
                    out=ph[:], lhsT=w1_sb[:], rhs=dxt_sb[:, bass.ts(g, group)],
                    start=True, stop=True,
                )
                h_s = smallp.tile([H_BOT, group], fp16, tag="h")
                nc.scalar.activation(
                    out=h_s[:], in_=ph[:],
                    func=mybir.ActivationFunctionType.Relu, bias=b1_sb[:],
                )
                pd = psmallp.tile([D, group], f32, tag="psmall")
                nc.tensor.matmul(
                    out=pd[:], lhsT=w2_sb[:], rhs=h_s[:], start=True, stop=True
                )
                dt_sb = smallp.tile([D, group], fp16, tag="dt")
                nc.scalar.activation(
                    out=dt_sb[:], in_=pd[:],
                    func=mybir.ActivationFunctionType.Identity, bias=b2_sb[:],
                )

                # ---- top MLP layer 1: fp8 DoubleRow accumulation ----
                po1 = po1p.tile([H_TOP, group], f32, tag="po1")
                for c in range(KC):
                    lhs = tw1dr_sb[:, bass.ts(c, 2 * H_TOP)].rearrange(
                        "p (r m) -> p r m", r=2
                    )
                    rhs = embt_g[:, c, :].bitcast(f8).rearrange(
                        "p (s r) -> p r s", r=2
                    )
                    if USE_DOUBLE_ROW:
                        nc.tensor.matmul(
                            out=po1[:], lhsT=lhs, rhs=rhs,
                            start=(c == 0), stop=False,
                            perf_mode=mybir.MatmulPerfMode.DoubleRow,
                        )
                    else:
                        for r in range(2):
                            nc.tensor.matmul(
                                out=po1[:], lhsT=lhs[:, r, :], rhs=rhs[:, r, :],
                                start=(c == 0 and r == 0), stop=False,
                            )
                # dense chunk carries the same S_T*S_W scale (folded into tw1d)
                nc.tensor.matmul(
                    out=po1[:], lhsT=tw1d_sb[:], rhs=dt_sb[:],
                    start=False, stop=True,
                )

                o1 = smallp.tile([H_TOP, group], fp16, tag="o1")
                nc.scalar.activation(
                    out=o1[:], in_=po1[:],
                    func=mybir.ActivationFunctionType.Relu,
                    bias=tb1_sb[:], scale=ALPHA,
                )
                plg = psmallp.tile([1, group], f32, tag="psmall")
                nc.tensor.matmul(
                    out=plg[:], lhsT=tw2_sb[:], rhs=o1[:], start=True, stop=True
                )
                nc.scalar.activation(
                    out=y_row[:, bass.ts(g, group)], in_=plg[:],
                    func=mybir.ActivationFunctionType.Sigmoid, bias=tb2_sb[:],
                )

            nc.sync.dma_start(out=y_d[:, :], in_=y_row[:])

    nc.compile()
    return nc


_NC_CACHE = {}


def _get_nc():
    if "nc" not in _NC_CACHE:
        _NC_CACHE["nc"] = build_kernel()
    return _NC_CACHE["nc"]


def make_in_maps(dense_x, sparse_x, tables, w1, b1, w2, b2, tw1, tb1, tw2, tb2):
    b_loc = B // N_CORES
    group = 512
    n_g = b_loc // group
    tpg = group // P

    tables_q = np.ascontiguousarray(
        (np.asarray(tables, np.float32).reshape(F * CARD, D) * S_T).astype(
            ml_dtypes.float8_e4m3
        )
    )

    tw1_f = np.asarray(tw1, np.float32)
    tw1s = np.zeros((KC * 2 * P, H_TOP), np.float32)
    tw1s[: F * D] = tw1_f[: F * D] * S_W
    # [k, m] -> [c, q, r, m] -> [q, c, r, m]; k = c*256 + q*2 + r
    tw1dr = (
        tw1s.reshape(KC, P, 2, H_TOP)
        .transpose(1, 0, 2, 3)
        .reshape(P, KC * 2 * H_TOP)
        .astype(ml_dtypes.float8_e4m3)
    )
    tw1d = np.ascontiguousarray(
        (tw1_f[F * D : F * D + D] * (S_T * S_W)).astype(np.float16)
    )

    dense_f = np.asarray(dense_x, np.float32)
    sparse_i = np.asarray(sparse_x, np.int64).astype(np.int32)
    foffs = (np.arange(F, dtype=np.int32) * CARD)[None, :]

    shared = {
        "tables": tables_q,
        "tw1dr": np.ascontiguousarray(tw1dr),
        "tw1d": tw1d,
        "w1": np.ascontiguousarray(np.asarray(w1, np.float16)),
        "b1": np.ascontiguousarray(np.asarray(b1, np.float32)),
        "w2": np.ascontiguousarray(np.asarray(w2, np.float16)),
        "b2": np.ascontiguousarray(np.asarray(b2, np.float32)),
        "tb1": np.ascontiguousarray(np.asarray(tb1, np.float32)),
        "tw2": np.ascontiguousarray(np.asarray(tw2, np.float16)),
        "tb2": np.ascontiguousarray(np.asarray(tb2, np.float32)),
    }

    in_maps = []
    for cidx in range(N_CORES):
        sl = slice(cidx * b_loc, (cidx + 1) * b_loc)
        comb = np.zeros((b_loc, FPAD), np.int32)
        comb[:, :F] = sparse_i[sl] + foffs
        comb = np.ascontiguousarray(
            comb.reshape(n_g, tpg, P, FPAD)
            .transpose(2, 0, 1, 3)
            .reshape(P, n_g * tpg * FPAD)
        )
        dxt = np.ascontiguousarray(dense_f[sl].T.astype(np.float16))
        m = dict(shared)
        m["comb"] = comb
        m["dxt"] = dxt
        in_maps.append(m)
    return in_maps


def kernel(**inputs):
    from concourse.bass_utils import run_bass_kernel_spmd

    nc = _get_nc()
    in_maps = make_in_maps(**inputs)
    res = run_bass_kernel_spmd(nc, in_maps, core_ids=list(range(N_CORES)))
    out = np.concatenate([r["y"].reshape(-1) for r in res.results])
    return out.reshape(B, 1).astype(np.float32)


# revision 17
# speedup vs baseline: 1.2114x; 1.0923x over previous
"""DLRM forward (embedding gather + tiny MLPs) as a Bass/Tile kernel on 8 trn2 cores.

Sharding: data-parallel over the batch. Each of the 8 cores gets B/8 = 2048 rows
of dense_x / sparse_x plus a full replica of the (read-only) embedding tables,
computes its batch shard end-to-end on device, and returns [1, 2048] sigmoid
outputs. The host only slices inputs and concatenates outputs.

v2 design (vs v1: fp16 tables + per-128-tile gathers + PE transposes):
  - tables quantized host-side to fp8e4 (x256 scale) -> 64B gather rows, half
    the HBM gather traffic; scale undone in the o1 activation (ALPHA).
  - flat gather indices (idx + f*CARD) precomputed on host, padded to 28
    rows/sample so each 128-sample frame is 1792B = 896 u16 (xbar-friendly).
  - one indirect DMA per 512-sample group (4 total) -> fewer SWDGE gens.
  - feature->partition transposes done by the DMA XBAR (dma_start
    transpose=True) on uint16 views: one instruction block-transposes
    [128 samples, 7x128 u16] -> [128 u16-feat, 7, 128 samples]; each u16
    carries an (even,odd) fp8 feature pair, which is exactly the DoubleRow
    matmul operand layout.
  - top-MLP layer 1 accumulated with fp8 DoubleRow matmuls (2 features per
    partition per pass), weights host-packed into [q, c, r, m] order.
  - dense_x host-transposed to [13, B] fp16 so the bottom MLP needs no PE
    transposes at all.
"""

import numpy as np
import ml_dtypes

import concourse.bass as bass
import concourse.mybir as mybir
import concourse.tile as tile
from concourse import bacc
from concourse.masks import make_identity

P = 128

# Problem constants (hardcoded per harness contract).
N_CORES = 8
B = 16384
F = 26
D = 64
DENSE = 13
CARD = 100000
H_BOT = 8
H_TOP = 16

FPAD = 28                 # gathered rows per sample (26 + 2 pad -> 1792B frames)
FRAME = FPAD * D          # 1792 fp8 per sample-frame
KC = FRAME // 2 // P      # 7 u16 feature chunks of 128

S_T = 256.0               # host scale on tables before fp8e4 quantization
S_W = 64.0                # host scale on tw1 emb rows before fp8e4 quantization
ALPHA = 1.0 / (S_T * S_W)

f32 = mybir.dt.float32
i32 = mybir.dt.int32
fp16 = mybir.dt.float16
u16 = mybir.dt.bfloat16  # 2-byte raw carrier for fp8 pairs (ldweights accepts fp types only)
f8 = mybir.dt.float8e4

USE_DOUBLE_ROW = True


def build_kernel(b_loc=B // N_CORES):
    group = 512
    n_g = b_loc // group
    tpg = group // P

    nc = bacc.Bacc("TRN2", target_bir_lowering=False)
    comb_d = nc.dram_tensor("comb", [P, n_g * tpg * FPAD], i32, kind="ExternalInput")
    tables_d = nc.dram_tensor("tables", [F * CARD, D], f8, kind="ExternalInput")
    dxt_d = nc.dram_tensor("dxt", [DENSE, b_loc], fp16, kind="ExternalInput")
    w1_d = nc.dram_tensor("w1", [DENSE, H_BOT], fp16, kind="ExternalInput")
    b1_d = nc.dram_tensor("b1", [H_BOT], f32, kind="ExternalInput")
    w2_d = nc.dram_tensor("w2", [H_BOT, D], fp16, kind="ExternalInput")
    b2_d = nc.dram_tensor("b2", [D], f32, kind="ExternalInput")
    tw1dr_d = nc.dram_tensor("tw1dr", [P, KC * 2 * H_TOP], f8, kind="ExternalInput")
    tw1d_d = nc.dram_tensor("tw1d", [D, H_TOP], fp16, kind="ExternalInput")
    tb1_d = nc.dram_tensor("tb1", [H_TOP], f32, kind="ExternalInput")
    tw2_d = nc.dram_tensor("tw2", [H_TOP, 1], fp16, kind="ExternalInput")
    tb2_d = nc.dram_tensor("tb2", [1], f32, kind="ExternalInput")
    y_d = nc.dram_tensor("y", [1, b_loc], f32, kind="ExternalOutput")

    with tile.TileContext(nc) as tc:
        with (
            tc.tile_pool(name="const", bufs=1) as cpool,
            tc.tile_pool(name="emb", bufs=2) as embp,
            tc.tile_pool(name="embT", bufs=2) as embtp,
            tc.tile_pool(name="small", bufs=2) as smallp,
            tc.tile_pool(name="po1", bufs=2, space="PSUM") as po1p,
            tc.tile_pool(name="psmall", bufs=2, space="PSUM") as psmallp,
            tc.tile_pool(name="ptr", bufs=2, space="PSUM") as ptrp,
        ):
            # ---- constants / inputs staged once ----
            comb_sb = cpool.tile([P, n_g * tpg * FPAD], i32)
            nc.sync.dma_start(out=comb_sb[:], in_=comb_d[:, :])
            dxt_sb = cpool.tile([DENSE, b_loc], fp16)
            nc.sync.dma_start(out=dxt_sb[:], in_=dxt_d[:, :])
            tw1dr_sb = cpool.tile([P, KC * 2 * H_TOP], f8)
            nc.sync.dma_start(out=tw1dr_sb[:], in_=tw1dr_d[:, :])
            tw1d_sb = cpool.tile([D, H_TOP], fp16)
            nc.sync.dma_start(out=tw1d_sb[:], in_=tw1d_d[:, :])
            tw2_sb = cpool.tile([H_TOP, 1], fp16)
            nc.sync.dma_start(out=tw2_sb[:], in_=tw2_d[:, :])
            w1_sb = cpool.tile([DENSE, H_BOT], fp16)
            nc.sync.dma_start(out=w1_sb[:], in_=w1_d[:, :])
            w2_sb = cpool.tile([H_BOT, D], fp16)
            nc.sync.dma_start(out=w2_sb[:], in_=w2_d[:, :])
            b1_sb = cpool.tile([H_BOT, 1], f32)
            nc.sync.dma_start(out=b1_sb[:], in_=b1_d[:, None])
            b2_sb = cpool.tile([D, 1], f32)
            nc.sync.dma_start(out=b2_sb[:], in_=b2_d[:, None])
            tb1_sb = cpool.tile([H_TOP, 1], f32)
            nc.sync.dma_start(out=tb1_sb[:], in_=tb1_d[:, None])
            tb2_sb = cpool.tile([1, 1], f32)
            nc.sync.dma_start(out=tb2_sb[:], in_=tb2_d[:, None])

            ident_u = cpool.tile([P, P], u16)
            make_identity(nc, ident_u[:])

            y_row = cpool.tile([1, b_loc], f32)

            for g in range(n_g):
                # ---- gather: one indirect DMA for the whole 512-sample group ----
                emb_g = embp.tile([P, tpg * FRAME], f8, tag="emb")
                nc.gpsimd.indirect_dma_start(
                    out=emb_g[:],
                    out_offset=None,
                    in_=tables_d[:, :],
                    in_offset=bass.IndirectOffsetOnAxis(
                        ap=comb_sb[:, bass.ts(g, tpg * FPAD)], axis=0
                    ),
                )

                # ---- feature->partition transpose via DMA XBAR on u16 views ----
                # in:  [128 samples, 896 u16]  (u16 = fp8 feature pair)
                # out: [128 u16-feat, 7 chunks, 128 samples]
                embt_g = embtp.tile([P, KC, group], u16, tag="embT")
                for j in range(tpg):
                    ej = emb_g[:, bass.ts(j, FRAME)].bitcast(u16)
                    if j < 2:
                        eng = nc.sync if j % 2 == 0 else nc.scalar
                        eng.dma_start(
                            out=embt_g[:, :, bass.ts(j, P)],
                            in_=ej,
                            transpose=True,
                        )
                    else:
                        ptr = ptrp.tile([P, KC * P], u16, tag="ptr")
                        for c in range(KC):
                            nc.tensor.transpose(
                                out=ptr[:, bass.ts(c, P)],
                                in_=ej[:, bass.ts(c, P)],
                                identity=ident_u[:],
                            )
                        nc.vector.tensor_copy(
                            out=embt_g[:, :, bass.ts(j, P)],
                            in_=ptr[:].rearrange("p (c s) -> p c s", c=KC),
                        )

                # ---- bottom MLP (host-transposed dense, fp16) ----
                ph = psmallp.tile([H_BOT, group], f32, tag="psmall")
                nc.tensor.matmul(
                    out=ph[:], lhsT=w1_sb[:], rhs=dxt_sb[:, bass.ts(g, group)],
                    start=True, stop=True,
                )
                h_s = smallp.tile([H_BOT, group], fp16, tag="h")
                nc.scalar.activation(
                    out=h_s[:], in_=ph[:],
                    func=mybir.ActivationFunctionType.Relu, bias=b1_sb[:],
                )
                pd = psmallp.tile([D, group], f32, tag="psmall")
                nc.tensor.matmul(
                    out=pd[:], lhsT=w2_sb[:], rhs=h_s[:], start=True, stop=True
                )
                dt_sb = smallp.tile([D, group], fp16, tag="dt")
                nc.scalar.activation(
                    out=dt_sb[:], in_=pd[:],
                    func=mybir.ActivationFunctionType.Identity, bias=b2_sb[:],
                )

                # ---- top MLP layer 1: fp8 DoubleRow accumulation ----
                po1 = po1p.tile([H_TOP, group], f32, tag="po1")
                for c in range(KC):
                    lhs = tw1dr_sb[:, bass.ts(c, 2 * H_TOP)].rearrange(
                        "p (r m) -> p r m", r=2
                    )
                    rhs = embt_g[:, c, :].bitcast(f8).rearrange(
                        "p (s r) -> p r s", r=2
                    )
                    if USE_DOUBLE_ROW:
                        nc.tensor.matmul(
                            out=po1[:], lhsT=lhs, rhs=rhs,
                            start=(c == 0), stop=False,
                            perf_mode=mybir.MatmulPerfMode.DoubleRow,
                        )
                    else:
                        for r in range(2):
                            nc.tensor.matmul(
                                out=po1[:], lhsT=lhs[:, r, :], rhs=rhs[:, r, :],
                                start=(c == 0 and r == 0), stop=False,
                            )
                # dense chunk carries the same S_T*S_W scale (folded into tw1d)
                nc.tensor.matmul(
                    out=po1[:], lhsT=tw1d_sb[:], rhs=dt_sb[:],
                    start=False, stop=True,
                )

                o1 = smallp.tile([H_TOP, group], fp16, tag="o1")
                nc.scalar.activation(
                    out=o1[:], in_=po1[:],
                    func=mybir.ActivationFunctionType.Relu,
                    bias=tb1_sb[:], scale=ALPHA,
                )
                plg = psmallp.tile([1, group], f32, tag="psmall")
                nc.tensor.matmul(
                    out=plg[:], lhsT=tw2_sb[:], rhs=o1[:], start=True, stop=True
                )
                nc.scalar.activation(
                    out=y_row[:, bass.ts(g, group)], in_=plg[:],
                    func=mybir.ActivationFunctionType.Sigmoid, bias=tb2_sb[:],
                )

            nc.sync.dma_start(out=y_d[:, :], in_=y_row[:])

    nc.compile()
    return nc


_NC_CACHE = {}


def _get_nc():
    if "nc" not in _NC_CACHE:
        _NC_CACHE["nc"] = build_kernel()
    return _NC_CACHE["nc"]


def make_in_maps(dense_x, sparse_x, tables, w1, b1, w2, b2, tw1, tb1, tw2, tb2):
    b_loc = B // N_CORES
    group = 512
    n_g = b_loc // group
    tpg = group // P

    tables_q = np.ascontiguousarray(
        (np.asarray(tables, np.float32).reshape(F * CARD, D) * S_T).astype(
            ml_dtypes.float8_e4m3
        )
    )

    tw1_f = np.asarray(tw1, np.float32)
    tw1s = np.zeros((KC * 2 * P, H_TOP), np.float32)
    tw1s[: F * D] = tw1_f[: F * D] * S_W
    # [k, m] -> [c, q, r, m] -> [q, c, r, m]; k = c*256 + q*2 + r
    tw1dr = (
        tw1s.reshape(KC, P, 2, H_TOP)
        .transpose(1, 0, 2, 3)
        .reshape(P, KC * 2 * H_TOP)
        .astype(ml_dtypes.float8_e4m3)
    )
    tw1d = np.ascontiguousarray(
        (tw1_f[F * D : F * D + D] * (S_T * S_W)).astype(np.float16)
    )

    dense_f = np.asarray(dense_x, np.float32)
    sparse_i = np.asarray(sparse_x, np.int64).astype(np.int32)
    foffs = (np.arange(F, dtype=np.int32) * CARD)[None, :]

    shared = {
        "tables": tables_q,
        "tw1dr": np.ascontiguousarray(tw1dr),
        "tw1d": tw1d,
        "w1": np.ascontiguousarray(np.asarray(w1, np.float16)),
        "b1": np.ascontiguousarray(np.asarray(b1, np.float32)),
        "w2": np.ascontiguousarray(np.asarray(w2, np.float16)),
        "b2": np.ascontiguousarray(np.asarray(b2, np.float32)),
        "tb1": np.ascontiguousarray(np.asarray(tb1, np.float32)),
        "tw2": np.ascontiguousarray(np.asarray(tw2, np.float16)),
        "tb2": np.ascontiguousarray(np.asarray(tb2, np.float32)),
    }

    in_maps = []
    for cidx in range(N_CORES):
        sl = slice(cidx * b_loc, (cidx + 1) * b_loc)
        comb = np.zeros((b_loc, FPAD), np.int32)
        comb[:, :F] = sparse_i[sl] + foffs
        comb = np.ascontiguousarray(
            comb.reshape(n_g, tpg, P, FPAD)
            .transpose(2, 0, 1, 3)
            .reshape(P, n_g * tpg * FPAD)
        )
        dxt = np.ascontiguousarray(dense_f[sl].T.astype(np.float16))
        m = dict(shared)
        m["comb"] = comb
        m["dxt"] = dxt
        in_maps.append(m)
    return in_maps


def kernel(**inputs):
    from concourse.bass_utils import run_bass_kernel_spmd

    nc = _get_nc()
    in_maps = make_in_maps(**inputs)
    res = run_bass_kernel_spmd(nc, in_maps, core_ids=list(range(N_CORES)))
    out = np.concatenate([r["y"].reshape(-1) for r in res.results])
    return out.reshape(B, 1).astype(np.float32)
